# revision 7
# baseline (speedup 1.0000x reference)
"""Causal multi-head attention (B=2, S=2048, H=1024, 16 heads) on 8 trn2 cores.

Sharding: tensor-parallel over heads. Each core owns 2 heads: the matching
128 rows of Wq/Wk/Wv (QKV output columns), attention for those heads, and
the matching 128 columns of Wp. Cores return their [2,2,2048,2048] attention
slice plus a full-shape partial output; the host concatenates attention
slices over the head axis and sums the partials (+ bp).

Per core, per batch element:
  1. QKV: stream xT chunks, matmul into QT/KT (head-dim on partitions) and
     VT; PE-transpose VT -> V [token, d].
  2. Scores: S = QT.T @ KT per head in row-packed head pairs (contraction 64),
     additive -1e30 causal masks on the diagonal block, fused exp+rowsum on
     ScalarE, normalize on VectorE, DMA out. Above-diagonal attn stays zero
     because output buffers arrive pre-zeroed.
  3. Transposed scores ST = KT.T @ QT, exp -> P^T tiles feeding PV matmuls
     directly (ctx^T accumulated in PSUM, heads column-packed).
  4. ctx^T normalized by rowsum reciprocals (DMA-broadcast via DRAM scratch),
     then the output projection -> partial out, DMA'd straight from PSUM.

Matmul precision modes per stage: 'split' = fp32r hi/lo 3-pass (full fp32
accuracy at 3 cyc/row), 'f32r' = single-pass rounded fp32 (~1.5e-4 rel),
'f32' = plain fp32 (exact, 4 cyc/row).
"""
import os
import sys

sys.path.insert(0, "/opt/trn_rl_repo")

import math
from contextlib import ExitStack

import numpy as np

from concourse import bacc, mybir, tile
from concourse.bass_utils import run_bass_kernel_spmd
from concourse.masks import make_identity

F32 = mybir.dt.float32
F32R = mybir.dt.float32r
EXP = mybir.ActivationFunctionType.Exp
ADD = mybir.AluOpType.add
SUB = mybir.AluOpType.subtract
MULT = mybir.AluOpType.mult

B, S, H = 2, 2048, 1024
NH, HD = 16, 64
NCORES = 8
NQT = S // 128    # 16 q-tiles
SCALE = 1.0 / math.sqrt(HD)
NEG = -1.0e30

# qkv/s: 'split' | 'f32r'     pv: 'f32r' | 'f32'    proj: 'split' | 'f32r' | 'f32'
CFG = dict(
    qkv=os.environ.get("CMGA_QKV", "split"),
    s=os.environ.get("CMGA_S", "split"),
    pv=os.environ.get("CMGA_PV", "f32"),
    proj=os.environ.get("CMGA_PROJ", "split"),
)


def _round_fp32r(x):
    """Round-to-nearest-even to fp32r (12 explicit mantissa bits).

    Bit-exact match of the hardware's fp32->fp32r downconversion."""
    u = np.ascontiguousarray(x).view(np.uint32)
    lsb = (u >> 12) & 1
    return ((u + 0x7FF + lsb) & 0xFFFFF000).view(np.float32)


def _split_hi_lo(x):
    hi = _round_fp32r(x)
    lo = _round_fp32r((x - hi).astype(np.float32))
    return hi, lo


def _s_mask_tiles():
    """Additive causal masks for the last 256 columns of an S row-span.

    variant 0 (even q-tile): cols 0-127 lower-triangular, cols 128-255 masked.
    variant 1 (odd q-tile): cols 0-127 valid, cols 128-255 lower-triangular."""
    r = np.arange(128)[:, None]
    c = np.arange(256)[None, :]
    m = np.zeros((2, 128, 256), np.float32)
    m[0][c > r] = NEG
    m[1][(c >= 128) & (c - 128 > r)] = NEG
    return m


def _t_mask_tiles():
    """Additive causal masks for the ST diagonal 512-block (k on partitions).

    variant v (= j mod 4): valid iff q_rel >= 128*v + k_rel."""
    r = np.arange(128)[:, None]
    c = np.arange(512)[None, :]
    m = np.zeros((4, 128, 512), np.float32)
    for v in range(4):
        m[v][c < 128 * v + r] = NEG
    return m


def build_bass(cfg=CFG):
    nc = bacc.Bacc("TRN2", target_bir_lowering=False, debug=False)

    def din(name, shape, dt=F32):
        return nc.dram_tensor(name, list(shape), dt, kind="ExternalInput").ap()

    def dout(name, shape, dt=F32):
        return nc.dram_tensor(name, list(shape), dt, kind="ExternalOutput").ap()

    qkv_split = cfg["qkv"] == "split"
    s_split = cfg["s"] == "split"
    pv_dt = F32 if cfg["pv"] == "f32" else F32R
    proj_split = cfg["proj"] == "split"
    proj_dt = F32 if cfg["proj"] == "f32" else F32R

    # all inputs host-pre-arranged partition-major so every DMA is a plain
    # contiguous [128, N] copy
    xt_hi = din("xt_hi", (B, 128, 8, S), F32R)
    xt_lo = din("xt_lo", (B, 128, 8, S), F32R) if qkv_split else None
    w_hi, w_lo = {}, {}
    for w in ("wq", "wk", "wv"):
        w_hi[w] = din(w + "_hi", (128, H), F32R)
        if qkv_split:
            w_lo[w] = din(w + "_lo", (128, H), F32R)
    wp_hi = din("wp_hi", (128, H), proj_dt)
    wp_lo = din("wp_lo", (128, H), proj_dt) if proj_split else None
    biases = {bn: din(bn, (128, 1)) for bn in ("bq", "bk", "bv")}
    mask_s = din("mask_s", (128, 512))
    mask_t = din("mask_t", (128, 2048))

    attn_o = dout("attn_o", (B, 2, S, S))
    out_o = dout("out_o", (B, S, H))

    TC = 512  # QKV token-chunk width
    NTC = S // TC

    with ExitStack() as ctx:
        tc = ctx.enter_context(tile.TileContext(nc))
        const = ctx.enter_context(tc.tile_pool(name="const", bufs=1))
        keep = ctx.enter_context(tc.tile_pool(name="keep", bufs=1))
        st2 = ctx.enter_context(tc.tile_pool(name="st2", bufs=2))
        st1 = ctx.enter_context(tc.tile_pool(name="st1", bufs=1))
        wk3 = ctx.enter_context(tc.tile_pool(name="wk3", bufs=3))
        work = ctx.enter_context(tc.tile_pool(name="work", bufs=2))
        sml = ctx.enter_context(tc.tile_pool(name="sml", bufs=4))
        psA = ctx.enter_context(tc.tile_pool(name="psA", bufs=1, space="PSUM"))
        psB = ctx.enter_context(tc.tile_pool(name="psB", bufs=2, space="PSUM"))
        dram = ctx.enter_context(tc.tile_pool(name="dram", bufs=1, space="DRAM"))

        # ---- constants (gpsimd/SWDGE ring: keeps the sync ring free for xt) ----
        ident = const.tile([128, 128], F32)
        make_identity(nc, ident[:])
        msk_s = const.tile([128, 512], F32)
        nc.gpsimd.dma_start(msk_s[:], mask_s)
        msk_t = const.tile([128, 2048], F32)
        nc.gpsimd.dma_start(msk_t[:], mask_t)
        bias_t = {}
        for bn in ("bq", "bk", "bv"):
            bias_t[bn] = const.tile([128, 1], F32, tag=bn, name=bn)
            nc.gpsimd.dma_start(bias_t[bn][:], biases[bn])
        wsb_hi, wsb_lo = {}, {}
        for w in ("wq", "wk", "wv"):
            wsb_hi[w] = const.tile([128, H], F32R, tag=f"{w}hi", name=f"{w}hi")
            nc.gpsimd.dma_start(wsb_hi[w][:], w_hi[w])
            if qkv_split:
                wsb_lo[w] = const.tile([128, H], F32R, tag=f"{w}lo", name=f"{w}lo")
                nc.gpsimd.dma_start(wsb_lo[w][:], w_lo[w])
        wpsb_hi = const.tile([128, H], proj_dt, tag="wphi", name="wpsb_hi")
        nc.gpsimd.dma_start(wpsb_hi[:], wp_hi)
        wpsb_lo = None
        if proj_split:
            wpsb_lo = const.tile([128, H], proj_dt, tag="wplo", name="wpsb_lo")
            nc.gpsimd.dma_start(wpsb_lo[:], wp_lo)

        recip_d = dram.tile([B, 2, S], F32, name="recip_d")

        for b in range(B):
            # ---------------- QKV projections ----------------
            qt_hi = keep.tile([128, S], F32R, tag="qt_hi", name="qt_hi")
            kt_hi = keep.tile([128, S], F32R, tag="kt_hi", name="kt_hi")
            qt_lo = keep.tile([128, S], F32R, tag="qt_lo", name="qt_lo") if s_split else None
            kt_lo = keep.tile([128, S], F32R, tag="kt_lo", name="kt_lo") if s_split else None
            v_sb = keep.tile([128, S], pv_dt, tag="v_sb", name="v_sb")
            for t in range(NTC):
                tsl = slice(TC * t, TC * t + TC)
                x_hi = st2.tile([128, 8, TC], F32R, tag="xhi", name="x_hi")
                nc.sync.dma_start(x_hi[:], xt_hi[b, :, :, tsl])
                x_lo = None
                if qkv_split:
                    x_lo = st1.tile([128, 8, TC], F32R, tag="xlo", name="x_lo")
                    nc.sync.dma_start(x_lo[:], xt_lo[b, :, :, tsl])
                vt = work.tile([128, TC], F32, tag="vt", name="vt")
                for w, bn in (("wq", "bq"), ("wk", "bk"), ("wv", "bv")):
                    wps = psB.tile([128, TC], F32, tag="mm", name="wps")
                    mms = []
                    for e in range(8):
                        esl = slice(128 * e, 128 * e + 128)
                        mms.append((wsb_hi[w][:, esl], x_hi[:, e]))
                        if qkv_split:
                            mms.append((wsb_hi[w][:, esl], x_lo[:, e]))
                            mms.append((wsb_lo[w][:, esl], x_hi[:, e]))
                    for idx, (lw, lx) in enumerate(mms):
                        nc.tensor.matmul(wps[:], lw, lx, start=(idx == 0),
                                         stop=(idx == len(mms) - 1))
                    bias = bias_t[bn][:]
                    if w == "wv":
                        nc.vector.tensor_scalar_add(vt[:], wps[:], bias)
                    else:
                        dst_hi = qt_hi if w == "wq" else kt_hi
                        nc.vector.tensor_scalar_add(dst_hi[:, tsl], wps[:], bias)
                        if s_split:
                            dst_lo = qt_lo if w == "wq" else kt_lo
                            nc.vector.scalar_tensor_tensor(
                                dst_lo[:, tsl], wps[:], bias,
                                dst_hi[:, tsl].bitcast(F32), op0=ADD, op1=SUB,
                            )
                # V = transpose(VT chunk): [128 d, 128 k] -> [128 k, 128 d]
                for u in range(TC // 128):
                    col = TC * t + 128 * u
                    tp = psB.tile([128, 128], F32, tag="mm", name="tp")
                    nc.tensor.transpose(tp[:], vt[:, 128 * u:128 * u + 128],
                                        ident[:])
                    nc.vector.tensor_copy(v_sb[:, col:col + 128], tp[:])

            # ---------------- S pass: scores -> attn out ----------------
            for i in range(NQT):
                W = 256 * ((i + 2) // 2)
                nsub = (W + 1023) // 1024
                pspan = [wk3.tile([128, 2048], F32, tag="pspan", name=f"pspan{h}")[:, :W]
                         for h in (0, 1)]
                rs = [sml.tile([128, 2], F32, tag=f"rs{h}", name=f"rs{h}") for h in (0, 1)]
                qsl = slice(128 * i, 128 * i + 128)
                for sub in range(nsub):
                    w0 = 1024 * sub
                    w1 = min(W, w0 + 1024)
                    sps = [psA.tile([128, 1024], F32, tag=f"s{h}", name=f"sps{h}")[:, :w1 - w0]
                           for h in (0, 1)]
                    c0 = w0
                    while c0 < w1:
                        cw = 512 if c0 + 512 <= w1 else 256
                        for h in (0, 1):
                            hsl = slice(64 * h, 64 * h + 64)
                            mms = [(qt_hi[hsl, qsl], kt_hi[hsl, c0:c0 + cw])]
                            if s_split:
                                mms += [(qt_hi[hsl, qsl], kt_lo[hsl, c0:c0 + cw]),
                                        (qt_lo[hsl, qsl], kt_hi[hsl, c0:c0 + cw])]
                            for idx, (lq, lk) in enumerate(mms):
                                nc.tensor.matmul(
                                    sps[h][:, c0 - w0:c0 - w0 + cw], lq, lk,
                                    start=(idx == 0), stop=(idx == len(mms) - 1),
                                )
                        c0 += cw
                    for h in (0, 1):
                        if sub == nsub - 1:
                            nc.vector.tensor_tensor(
                                sps[h][:, W - 256 - w0:W - w0],
                                sps[h][:, W - 256 - w0:W - w0],
                                msk_s[:, 256 * (i % 2):256 * (i % 2) + 256], ADD,
                            )
                        nc.scalar.activation(
                            pspan[h][:, w0:w1], sps[h][:], EXP,
                            scale=SCALE, accum_out=rs[h][:, sub:sub + 1],
                        )
                for h in (0, 1):
                    rec = sml.tile([128, 1], F32, tag=f"rec{h}", name=f"rec{h}")
                    if nsub == 2:
                        tot = sml.tile([128, 1], F32, tag=f"tot{h}", name=f"tot{h}")
                        nc.vector.tensor_tensor(tot[:], rs[h][:, 0:1],
                                                rs[h][:, 1:2], ADD)
                        nc.vector.reciprocal(rec[:], tot[:])
                    else:
                        nc.vector.reciprocal(rec[:], rs[h][:, 0:1])
                    nc.vector.tensor_scalar_mul(pspan[h][:], pspan[h][:], rec[:])
                    nc.scalar.dma_start(attn_o[b, h, qsl, 0:W], pspan[h][:])
                    nc.gpsimd.dma_start(recip_d[b, h, qsl], rec[:, 0])

            # ---------------- ST + PV: ctx^T accumulation ----------------
            ctx_f = keep.tile([128, S], F32, tag="ctx_f", name="ctx_f")
            for wave in range(2):
                qlo = 1024 * wave
                jmax = 8 if wave == 0 else 16
                cps = [psB.tile([128, 512], F32, tag="ctx", name=f"cps{_g}") for _g in range(2)]
                for j in range(jmax):
                    qstart = max(qlo, 512 * (j // 4))
                    Wj = qlo + 1024 - qstart
                    ksl = slice(128 * j, 128 * j + 128)
                    stps = [psA.tile([128, 1024], F32, tag=f"s{h}", name=f"stps{h}")[:, :Wj]
                            for h in (0, 1)]
                    for c0 in range(0, Wj, 512):
                        qb = slice(qstart + c0, qstart + c0 + 512)
                        for h in (0, 1):
                            hsl = slice(64 * h, 64 * h + 64)
                            mms = [(kt_hi[hsl, ksl], qt_hi[hsl, qb])]
                            if s_split:
                                mms += [(kt_hi[hsl, ksl], qt_lo[hsl, qb]),
                                        (kt_lo[hsl, ksl], qt_hi[hsl, qb])]
                            for idx, (lk, lq) in enumerate(mms):
                                nc.tensor.matmul(
                                    stps[h][:, c0:c0 + 512], lk, lq,
                                    start=(idx == 0), stop=(idx == len(mms) - 1),
                                )
                    diag_q = 512 * (j // 4)
                    for h in (0, 1):
                        if diag_q >= qlo:
                            off = diag_q - qstart
                            nc.vector.tensor_tensor(
                                stps[h][:, off:off + 512],
                                stps[h][:, off:off + 512],
                                msk_t[:, 512 * (j % 4):512 * (j % 4) + 512], ADD,
                            )
                        pt = wk3.tile([128, 1024], pv_dt, tag="pt", name=f"pt{h}")[:, :Wj]
                        nc.scalar.activation(pt[:], stps[h][:], EXP, scale=SCALE)
                        for c0 in range(0, Wj, 512):
                            g = (qstart + c0) // 512
                            gg = g - 2 * wave
                            nc.tensor.matmul(
                                cps[gg][64 * h:64 * h + 64, :],
                                v_sb[:, 128 * j + 64 * h:128 * j + 64 * h + 64],
                                pt[:, c0:c0 + 512],
                                start=(j == 0), stop=(j == 4 * g + 3),
                                skip_group_check=True,
                            )
                for gg in range(2):
                    g = 2 * wave + gg
                    gsl = slice(512 * g, 512 * g + 512)
                    bc = work.tile([128, 512], F32, tag="bcast", name="bc")
                    for h in (0, 1):
                        nc.gpsimd.dma_start(
                            bc[64 * h:64 * h + 64, :],
                            recip_d[b, h, gsl].partition_broadcast(64),
                        )
                    nc.vector.tensor_tensor(ctx_f[:, gsl], cps[gg][:], bc[:], MULT)

            # ---------------- output projection ----------------
            for t in range(16):
                tsl = slice(128 * t, 128 * t + 128)
                if proj_split:
                    c_hi = sml.tile([128, 128], proj_dt, tag="c_hi", name="c_hi")
                    c_lo = sml.tile([128, 128], proj_dt, tag="c_lo", name="c_lo")
                    nc.vector.tensor_copy(c_hi[:], ctx_f[:, tsl])
                    nc.vector.tensor_tensor(c_lo[:], ctx_f[:, tsl],
                                            c_hi[:].bitcast(F32), SUB)
                elif proj_dt is F32R:
                    c_hi = sml.tile([128, 128], proj_dt, tag="c_hi", name="c_hi")
                    nc.vector.tensor_copy(c_hi[:], ctx_f[:, tsl])
                    c_lo = None
                else:
                    c_hi, c_lo = ctx_f[:, tsl], None
                for n in range(2):
                    nsl = slice(512 * n, 512 * n + 512)
                    ops = psB.tile([128, 512], F32, tag="mm", name="ops")
                    mms = [(c_hi if isinstance(c_hi, tile.Tile) else c_hi,
                            wpsb_hi[:, nsl])]
                    if proj_split:
                        mms += [(c_hi, wpsb_lo[:, nsl]), (c_lo, wpsb_hi[:, nsl])]
                    for idx, (lc, lw) in enumerate(mms):
                        lc_ap = lc[:] if hasattr(lc, "tile") else lc
                        nc.tensor.matmul(ops[:], lc_ap, lw, start=(idx == 0),
                                         stop=(idx == len(mms) - 1))
                    ostg = sml.tile([128, 512], F32, tag="ostg", name="ostg")
                    nc.vector.tensor_copy(ostg[:], ops[:])
                    nc.sync.dma_start(out_o[b, tsl, nsl], ostg[:])

    nc.compile()
    return nc


_CACHE = {}


def _get_nc():
    key = tuple(sorted(CFG.items()))
    if key not in _CACHE:
        _CACHE[key] = build_bass(CFG)
    return _CACHE[key]


def prep_inputs(x, Wq, bq, Wk, bk, Wv, bv, Wp, bp):
    """Build the 8 per-core input maps."""
    cfg = CFG
    # xt arranged [B, 128, 8, S]: xt[b, p, e, t] = x[b, t, 128*e + p]
    xt = np.ascontiguousarray(
        np.asarray(x, np.float32).reshape(B, S, 8, 128).transpose(0, 3, 2, 1))
    if cfg["qkv"] == "split":
        xt_hi, xt_lo = _split_hi_lo(xt)
    else:
        xt_hi, xt_lo = _round_fp32r(xt), None
    mask_s = np.ascontiguousarray(
        _s_mask_tiles().transpose(1, 0, 2).reshape(128, 512))
    mask_t = np.ascontiguousarray(
        _t_mask_tiles().transpose(1, 0, 2).reshape(128, 2048))
    maps = []
    for c in range(NCORES):
        rows = slice(128 * c, 128 * c + 128)
        m = {"xt_hi": xt_hi, "mask_s": mask_s, "mask_t": mask_t}
        if xt_lo is not None:
            m["xt_lo"] = xt_lo
        for nm, W in (("wq", Wq), ("wk", Wk), ("wv", Wv)):
            # [128, 1024] partition-major: w[p, 128*e + m] = W[rows][128*e + p? no:
            # wsb[p, e*128 + m] = W.T[e*128 + p, m] = W[rows][m_row...] — built as
            # W[rows].T reshaped (8,128,128) then [p, e, m]
            wt = np.asarray(W, np.float32)[rows].T  # [1024, 128]
            arr = np.ascontiguousarray(
                wt.reshape(8, 128, 128).transpose(1, 0, 2).reshape(128, H))
            if cfg["qkv"] == "split":
                hi, lo = _split_hi_lo(arr)
                m[nm + "_hi"] = hi
                m[nm + "_lo"] = lo
            else:
                m[nm + "_hi"] = _round_fp32r(arr)
        wpt = np.ascontiguousarray(np.asarray(Wp, np.float32)[:, rows].T)  # [128,1024]
        if cfg["proj"] == "split":
            m["wp_hi"], m["wp_lo"] = _split_hi_lo(wpt)
        elif cfg["proj"] == "f32r":
            m["wp_hi"] = _round_fp32r(wpt)
        else:
            m["wp_hi"] = wpt
        for nm, bvec in (("bq", bq), ("bk", bk), ("bv", bv)):
            m[nm] = np.ascontiguousarray(
                np.asarray(bvec, np.float32)[rows].reshape(128, 1))
        maps.append(m)
    return maps


def run(inputs, trace=False):
    nc = _get_nc()
    maps = prep_inputs(**inputs)
    res = run_bass_kernel_spmd(nc, maps, list(range(NCORES)), trace=trace)
    attn = np.empty((B, NH, S, S), np.float32)
    acc = np.zeros((B, S, H), np.float64)
    for c in range(NCORES):
        r = res.results[c]
        attn[:, 2 * c:2 * c + 2] = r["attn_o"]
        acc += r["out_o"]
    out = (acc + np.asarray(inputs["bp"], np.float64)).astype(np.float32)
    return (out, attn), res


def kernel(**inputs):
    (out, attn), _ = run(inputs, trace=False)
    return (out, attn)


# revision 8
# speedup vs baseline: 1.2379x; 1.2379x over previous
"""Causal multi-head attention (B=2, S=2048, H=1024, 16 heads) on 8 trn2 cores.

Sharding: tensor-parallel over heads. Each core owns 2 heads: the matching
128 rows of Wq/Wk/Wv (QKV output columns), attention for those heads, and
the matching 128 columns of Wp. Cores return their [2,2,2048,2048] attention
slice plus a full-shape partial output; the host concatenates attention
slices over the head axis and sums the partials (+ bp).

Per core, per batch element:
  1. QKV: stream xT chunks, matmul into QT/KT (head-dim on partitions) and
     VT; PE-transpose VT -> V [token, d].
  2. Scores: S = QT.T @ KT per head in row-packed head pairs (contraction 64),
     additive -1e30 causal masks on the diagonal block, fused exp+rowsum on
     ScalarE, normalize on VectorE, DMA out. Above-diagonal attn stays zero
     because output buffers arrive pre-zeroed.
  3. Transposed scores ST = KT.T @ QT, exp -> P^T tiles feeding PV matmuls
     directly (ctx^T accumulated in PSUM, heads column-packed).
  4. ctx^T normalized by rowsum reciprocals (DMA-broadcast via DRAM scratch),
     then the output projection -> partial out, DMA'd straight from PSUM.

Matmul precision modes per stage: 'split' = fp32r hi/lo 3-pass (full fp32
accuracy at 3 cyc/row), 'f32r' = single-pass rounded fp32 (~1.5e-4 rel),
'f32' = plain fp32 (exact, 4 cyc/row).
"""
import os
import sys

sys.path.insert(0, "/opt/trn_rl_repo")

import math
from contextlib import ExitStack

import numpy as np

from concourse import bacc, mybir, tile
from concourse.bass_utils import run_bass_kernel_spmd
from concourse.masks import make_identity

F32 = mybir.dt.float32
F32R = mybir.dt.float32r
EXP = mybir.ActivationFunctionType.Exp
ADD = mybir.AluOpType.add
SUB = mybir.AluOpType.subtract
MULT = mybir.AluOpType.mult

B, S, H = 2, 2048, 1024
NH, HD = 16, 64
NCORES = 8
NQT = S // 128    # 16 q-tiles
SCALE = 1.0 / math.sqrt(HD)
NEG = -1.0e30

# qkv/s: 'split' | 'f32r'     pv: 'f32r' | 'f32'    proj: 'split' | 'f32r' | 'f32'
CFG = dict(
    qkv=os.environ.get("CMGA_QKV", "split"),
    s=os.environ.get("CMGA_S", "split"),
    pv=os.environ.get("CMGA_PV", "f32"),
    proj=os.environ.get("CMGA_PROJ", "split"),
)


def _round_fp32r(x):
    """Round-to-nearest-even to fp32r (12 explicit mantissa bits).

    Bit-exact match of the hardware's fp32->fp32r downconversion."""
    u = np.ascontiguousarray(x).view(np.uint32)
    lsb = (u >> 12) & 1
    return ((u + 0x7FF + lsb) & 0xFFFFF000).view(np.float32)


def _split_hi_lo(x):
    hi = _round_fp32r(x)
    lo = _round_fp32r((x - hi).astype(np.float32))
    return hi, lo


def _s_mask_tiles():
    """Additive causal masks for the last 256 columns of an S row-span.

    variant 0 (even q-tile): cols 0-127 lower-triangular, cols 128-255 masked.
    variant 1 (odd q-tile): cols 0-127 valid, cols 128-255 lower-triangular."""
    r = np.arange(128)[:, None]
    c = np.arange(256)[None, :]
    m = np.zeros((2, 128, 256), np.float32)
    m[0][c > r] = NEG
    m[1][(c >= 128) & (c - 128 > r)] = NEG
    return m


def _t_mask_tiles():
    """Additive causal masks for the ST diagonal 512-block (k on partitions).

    variant v (= j mod 4): valid iff q_rel >= 128*v + k_rel."""
    r = np.arange(128)[:, None]
    c = np.arange(512)[None, :]
    m = np.zeros((4, 128, 512), np.float32)
    for v in range(4):
        m[v][c < 128 * v + r] = NEG
    return m


def build_bass(cfg=CFG):
    nc = bacc.Bacc("TRN2", target_bir_lowering=False, debug=False)

    def din(name, shape, dt=F32):
        return nc.dram_tensor(name, list(shape), dt, kind="ExternalInput").ap()

    def dout(name, shape, dt=F32):
        return nc.dram_tensor(name, list(shape), dt, kind="ExternalOutput").ap()

    qkv_split = cfg["qkv"] == "split"
    s_split = cfg["s"] == "split"
    pv_dt = F32 if cfg["pv"] == "f32" else F32R
    proj_split = cfg["proj"] == "split"
    proj_dt = F32 if cfg["proj"] == "f32" else F32R

    # all inputs host-pre-arranged partition-major so every DMA is a plain
    # contiguous [128, N] copy
    xt_hi = din("xt_hi", (B, 128, 8, S), F32R)
    xt_lo = din("xt_lo", (B, 128, 8, S), F32R) if qkv_split else None
    w_hi, w_lo = {}, {}
    for w in ("wq", "wk", "wv"):
        w_hi[w] = din(w + "_hi", (128, H), F32R)
        if qkv_split:
            w_lo[w] = din(w + "_lo", (128, H), F32R)
    wp_hi = din("wp_hi", (128, H), proj_dt)
    wp_lo = din("wp_lo", (128, H), proj_dt) if proj_split else None
    biases = {bn: din(bn, (128, 1)) for bn in ("bq", "bk", "bv")}
    mask_s = din("mask_s", (128, 512))
    mask_t = din("mask_t", (128, 2048))

    attn_o = dout("attn_o", (B, 2, S, S))
    out_o = dout("out_o", (B, S, H))

    TC = 512  # QKV token-chunk width
    NTC = S // TC

    with ExitStack() as ctx:
        tc = ctx.enter_context(tile.TileContext(nc))
        const = ctx.enter_context(tc.tile_pool(name="const", bufs=1))
        keep = ctx.enter_context(tc.tile_pool(name="keep", bufs=1))
        st2 = ctx.enter_context(tc.tile_pool(name="st2", bufs=2))
        st1 = ctx.enter_context(tc.tile_pool(name="st1", bufs=1))
        wk3 = ctx.enter_context(tc.tile_pool(name="wk3", bufs=3))
        work = ctx.enter_context(tc.tile_pool(name="work", bufs=2))
        sml = ctx.enter_context(tc.tile_pool(name="sml", bufs=4))
        psA = ctx.enter_context(tc.tile_pool(name="psA", bufs=1, space="PSUM"))
        psB = ctx.enter_context(tc.tile_pool(name="psB", bufs=2, space="PSUM"))
        dram = ctx.enter_context(tc.tile_pool(name="dram", bufs=1, space="DRAM"))

        # ---- constants (gpsimd/SWDGE ring: keeps the sync ring free for xt) ----
        ident = const.tile([128, 128], F32)
        make_identity(nc, ident[:])
        msk_s = const.tile([128, 512], F32)
        nc.gpsimd.dma_start(msk_s[:], mask_s)
        msk_t = const.tile([128, 2048], F32)
        nc.gpsimd.dma_start(msk_t[:], mask_t)
        bias_t = {}
        for bn in ("bq", "bk", "bv"):
            bias_t[bn] = const.tile([128, 1], F32, tag=bn, name=bn)
            nc.gpsimd.dma_start(bias_t[bn][:], biases[bn])
        wsb_hi, wsb_lo = {}, {}
        for w in ("wq", "wk", "wv"):
            wsb_hi[w] = const.tile([128, H], F32R, tag=f"{w}hi", name=f"{w}hi")
            nc.gpsimd.dma_start(wsb_hi[w][:], w_hi[w])
            if qkv_split:
                wsb_lo[w] = const.tile([128, H], F32R, tag=f"{w}lo", name=f"{w}lo")
                nc.gpsimd.dma_start(wsb_lo[w][:], w_lo[w])
        wpsb_hi = const.tile([128, H], proj_dt, tag="wphi", name="wpsb_hi")
        nc.gpsimd.dma_start(wpsb_hi[:], wp_hi)
        wpsb_lo = None
        if proj_split:
            wpsb_lo = const.tile([128, H], proj_dt, tag="wplo", name="wpsb_lo")
            nc.gpsimd.dma_start(wpsb_lo[:], wp_lo)

        recip_d = dram.tile([B, 2, S], F32, name="recip_d")

        for b in range(B):
            # ---------------- QKV projections ----------------
            qt_hi = keep.tile([128, S], F32R, tag="qt_hi", name="qt_hi")
            kt_hi = keep.tile([128, S], F32R, tag="kt_hi", name="kt_hi")
            qt_lo = keep.tile([128, S], F32R, tag="qt_lo", name="qt_lo") if s_split else None
            kt_lo = keep.tile([128, S], F32R, tag="kt_lo", name="kt_lo") if s_split else None
            v_sb = keep.tile([128, S], pv_dt, tag="v_sb", name="v_sb")
            for t in range(NTC):
                tsl = slice(TC * t, TC * t + TC)
                x_hi = st2.tile([128, 8, TC], F32R, tag="xhi", name="x_hi")
                nc.sync.dma_start(x_hi[:], xt_hi[b, :, :, tsl])
                x_lo = None
                if qkv_split:
                    x_lo = st1.tile([128, 8, TC], F32R, tag="xlo", name="x_lo")
                    nc.sync.dma_start(x_lo[:], xt_lo[b, :, :, tsl])
                vt = work.tile([128, TC], F32, tag="vt", name="vt")
                for w, bn in (("wq", "bq"), ("wk", "bk"), ("wv", "bv")):
                    wps = psB.tile([128, TC], F32, tag="mm", name="wps")
                    mms = []
                    for e in range(8):
                        esl = slice(128 * e, 128 * e + 128)
                        mms.append((wsb_hi[w][:, esl], x_hi[:, e]))
                        if qkv_split:
                            mms.append((wsb_hi[w][:, esl], x_lo[:, e]))
                            mms.append((wsb_lo[w][:, esl], x_hi[:, e]))
                    for idx, (lw, lx) in enumerate(mms):
                        nc.tensor.matmul(wps[:], lw, lx, start=(idx == 0),
                                         stop=(idx == len(mms) - 1))
                    bias = bias_t[bn][:]
                    if w == "wv":
                        nc.vector.tensor_scalar_add(vt[:], wps[:], bias)
                    else:
                        dst_hi = qt_hi if w == "wq" else kt_hi
                        nc.vector.tensor_scalar_add(dst_hi[:, tsl], wps[:], bias)
                        if s_split:
                            dst_lo = qt_lo if w == "wq" else kt_lo
                            nc.vector.scalar_tensor_tensor(
                                dst_lo[:, tsl], wps[:], bias,
                                dst_hi[:, tsl].bitcast(F32), op0=ADD, op1=SUB,
                            )
                # V = transpose(VT chunk): [128 d, 128 k] -> [128 k, 128 d]
                for u in range(TC // 128):
                    col = TC * t + 128 * u
                    tp = psB.tile([128, 128], F32, tag="mm", name="tp")
                    nc.tensor.transpose(tp[:], vt[:, 128 * u:128 * u + 128],
                                        ident[:])
                    nc.vector.tensor_copy(v_sb[:, col:col + 128], tp[:])

            # ---------------- S pass: scores -> attn out ----------------
            for i in range(NQT):
                W = 256 * ((i + 2) // 2)
                nsub = (W + 1023) // 1024
                pspan = [wk3.tile([128, 2048], F32, tag="pspan", name=f"pspan{h}")[:, :W]
                         for h in (0, 1)]
                rs = [sml.tile([128, 2], F32, tag=f"rs{h}", name=f"rs{h}") for h in (0, 1)]
                qsl = slice(128 * i, 128 * i + 128)
                for sub in range(nsub):
                    w0 = 1024 * sub
                    w1 = min(W, w0 + 1024)
                    sps = [psA.tile([128, 1024], F32, tag=f"s{h}", name=f"sps{h}")[:, :w1 - w0]
                           for h in (0, 1)]
                    c0 = w0
                    while c0 < w1:
                        cw = 512 if c0 + 512 <= w1 else 256
                        for h in (0, 1):
                            hsl = slice(64 * h, 64 * h + 64)
                            mms = [(qt_hi[hsl, qsl], kt_hi[hsl, c0:c0 + cw])]
                            if s_split:
                                mms += [(qt_hi[hsl, qsl], kt_lo[hsl, c0:c0 + cw]),
                                        (qt_lo[hsl, qsl], kt_hi[hsl, c0:c0 + cw])]
                            for idx, (lq, lk) in enumerate(mms):
                                nc.tensor.matmul(
                                    sps[h][:, c0 - w0:c0 - w0 + cw], lq, lk,
                                    start=(idx == 0), stop=(idx == len(mms) - 1),
                                )
                        c0 += cw
                    for h in (0, 1):
                        if sub == nsub - 1:
                            nc.vector.tensor_tensor(
                                sps[h][:, W - 256 - w0:W - w0],
                                sps[h][:, W - 256 - w0:W - w0],
                                msk_s[:, 256 * (i % 2):256 * (i % 2) + 256], ADD,
                            )
                        nc.scalar.activation(
                            pspan[h][:, w0:w1], sps[h][:], EXP,
                            scale=SCALE, accum_out=rs[h][:, sub:sub + 1],
                        )
                for h in (0, 1):
                    rec = sml.tile([128, 1], F32, tag=f"rec{h}", name=f"rec{h}")
                    if nsub == 2:
                        tot = sml.tile([128, 1], F32, tag=f"tot{h}", name=f"tot{h}")
                        nc.vector.tensor_tensor(tot[:], rs[h][:, 0:1],
                                                rs[h][:, 1:2], ADD)
                        nc.vector.reciprocal(rec[:], tot[:])
                    else:
                        nc.vector.reciprocal(rec[:], rs[h][:, 0:1])
                    nc.vector.tensor_scalar_mul(pspan[h][:], pspan[h][:], rec[:])
                    nc.gpsimd.dma_start(attn_o[b, h, qsl, 0:W], pspan[h][:])
                    nc.gpsimd.dma_start(recip_d[b, h, qsl], rec[:, 0])

            # ---------------- ST + PV: ctx^T accumulation ----------------
            ctx_f = keep.tile([128, S], F32, tag="ctx_f", name="ctx_f")
            for wave in range(2):
                qlo = 1024 * wave
                jmax = 8 if wave == 0 else 16
                cps = [psB.tile([128, 512], F32, tag="ctx", name=f"cps{_g}") for _g in range(2)]
                for j in range(jmax):
                    qstart = max(qlo, 512 * (j // 4))
                    Wj = qlo + 1024 - qstart
                    ksl = slice(128 * j, 128 * j + 128)
                    stps = [psA.tile([128, 1024], F32, tag=f"s{h}", name=f"stps{h}")[:, :Wj]
                            for h in (0, 1)]
                    for c0 in range(0, Wj, 512):
                        qb = slice(qstart + c0, qstart + c0 + 512)
                        for h in (0, 1):
                            hsl = slice(64 * h, 64 * h + 64)
                            mms = [(kt_hi[hsl, ksl], qt_hi[hsl, qb])]
                            if s_split:
                                mms += [(kt_hi[hsl, ksl], qt_lo[hsl, qb]),
                                        (kt_lo[hsl, ksl], qt_hi[hsl, qb])]
                            for idx, (lk, lq) in enumerate(mms):
                                nc.tensor.matmul(
                                    stps[h][:, c0:c0 + 512], lk, lq,
                                    start=(idx == 0), stop=(idx == len(mms) - 1),
                                )
                    diag_q = 512 * (j // 4)
                    for h in (0, 1):
                        if diag_q >= qlo:
                            off = diag_q - qstart
                            nc.vector.tensor_tensor(
                                stps[h][:, off:off + 512],
                                stps[h][:, off:off + 512],
                                msk_t[:, 512 * (j % 4):512 * (j % 4) + 512], ADD,
                            )
                        pt = wk3.tile([128, 1024], pv_dt, tag="pt", name=f"pt{h}")[:, :Wj]
                        nc.scalar.activation(pt[:], stps[h][:], EXP, scale=SCALE)
                        for c0 in range(0, Wj, 512):
                            g = (qstart + c0) // 512
                            gg = g - 2 * wave
                            nc.tensor.matmul(
                                cps[gg][64 * h:64 * h + 64, :],
                                v_sb[:, 128 * j + 64 * h:128 * j + 64 * h + 64],
                                pt[:, c0:c0 + 512],
                                start=(j == 0), stop=(j == 4 * g + 3),
                                skip_group_check=True,
                            )
                for gg in range(2):
                    g = 2 * wave + gg
                    gsl = slice(512 * g, 512 * g + 512)
                    bc = work.tile([128, 512], F32, tag="bcast", name="bc")
                    for h in (0, 1):
                        nc.gpsimd.dma_start(
                            bc[64 * h:64 * h + 64, :],
                            recip_d[b, h, gsl].partition_broadcast(64),
                        )
                    nc.vector.tensor_tensor(ctx_f[:, gsl], cps[gg][:], bc[:], MULT)

            # ---------------- output projection ----------------
            for t in range(16):
                tsl = slice(128 * t, 128 * t + 128)
                if proj_split:
                    c_hi = sml.tile([128, 128], proj_dt, tag="c_hi", name="c_hi")
                    c_lo = sml.tile([128, 128], proj_dt, tag="c_lo", name="c_lo")
                    nc.vector.tensor_copy(c_hi[:], ctx_f[:, tsl])
                    nc.vector.tensor_tensor(c_lo[:], ctx_f[:, tsl],
                                            c_hi[:].bitcast(F32), SUB)
                elif proj_dt is F32R:
                    c_hi = sml.tile([128, 128], proj_dt, tag="c_hi", name="c_hi")
                    nc.vector.tensor_copy(c_hi[:], ctx_f[:, tsl])
                    c_lo = None
                else:
                    c_hi, c_lo = ctx_f[:, tsl], None
                for n in range(2):
                    nsl = slice(512 * n, 512 * n + 512)
                    ops = psB.tile([128, 512], F32, tag="mm", name="ops")
                    mms = [(c_hi if isinstance(c_hi, tile.Tile) else c_hi,
                            wpsb_hi[:, nsl])]
                    if proj_split:
                        mms += [(c_hi, wpsb_lo[:, nsl]), (c_lo, wpsb_hi[:, nsl])]
                    for idx, (lc, lw) in enumerate(mms):
                        lc_ap = lc[:] if hasattr(lc, "tile") else lc
                        nc.tensor.matmul(ops[:], lc_ap, lw, start=(idx == 0),
                                         stop=(idx == len(mms) - 1))
                    ostg = sml.tile([128, 512], F32, tag="ostg", name="ostg")
                    nc.vector.tensor_copy(ostg[:], ops[:])
                    nc.sync.dma_start(out_o[b, tsl, nsl], ostg[:])

    nc.compile()
    return nc


_CACHE = {}


def _get_nc():
    key = tuple(sorted(CFG.items()))
    if key not in _CACHE:
        _CACHE[key] = build_bass(CFG)
    return _CACHE[key]


def prep_inputs(x, Wq, bq, Wk, bk, Wv, bv, Wp, bp):
    """Build the 8 per-core input maps."""
    cfg = CFG
    # xt arranged [B, 128, 8, S]: xt[b, p, e, t] = x[b, t, 128*e + p]
    xt = np.ascontiguousarray(
        np.asarray(x, np.float32).reshape(B, S, 8, 128).transpose(0, 3, 2, 1))
    if cfg["qkv"] == "split":
        xt_hi, xt_lo = _split_hi_lo(xt)
    else:
        xt_hi, xt_lo = _round_fp32r(xt), None
    mask_s = np.ascontiguousarray(
        _s_mask_tiles().transpose(1, 0, 2).reshape(128, 512))
    mask_t = np.ascontiguousarray(
        _t_mask_tiles().transpose(1, 0, 2).reshape(128, 2048))
    maps = []
    for c in range(NCORES):
        rows = slice(128 * c, 128 * c + 128)
        m = {"xt_hi": xt_hi, "mask_s": mask_s, "mask_t": mask_t}
        if xt_lo is not None:
            m["xt_lo"] = xt_lo
        for nm, W in (("wq", Wq), ("wk", Wk), ("wv", Wv)):
            # [128, 1024] partition-major: w[p, 128*e + m] = W[rows][128*e + p? no:
            # wsb[p, e*128 + m] = W.T[e*128 + p, m] = W[rows][m_row...] — built as
            # W[rows].T reshaped (8,128,128) then [p, e, m]
            wt = np.asarray(W, np.float32)[rows].T  # [1024, 128]
            arr = np.ascontiguousarray(
                wt.reshape(8, 128, 128).transpose(1, 0, 2).reshape(128, H))
            if cfg["qkv"] == "split":
                hi, lo = _split_hi_lo(arr)
                m[nm + "_hi"] = hi
                m[nm + "_lo"] = lo
            else:
                m[nm + "_hi"] = _round_fp32r(arr)
        wpt = np.ascontiguousarray(np.asarray(Wp, np.float32)[:, rows].T)  # [128,1024]
        if cfg["proj"] == "split":
            m["wp_hi"], m["wp_lo"] = _split_hi_lo(wpt)
        elif cfg["proj"] == "f32r":
            m["wp_hi"] = _round_fp32r(wpt)
        else:
            m["wp_hi"] = wpt
        for nm, bvec in (("bq", bq), ("bk", bk), ("bv", bv)):
            m[nm] = np.ascontiguousarray(
                np.asarray(bvec, np.float32)[rows].reshape(128, 1))
        maps.append(m)
    return maps


def run(inputs, trace=False):
    nc = _get_nc()
    maps = prep_inputs(**inputs)
    res = run_bass_kernel_spmd(nc, maps, list(range(NCORES)), trace=trace)
    attn = np.empty((B, NH, S, S), np.float32)
    acc = np.zeros((B, S, H), np.float64)
    for c in range(NCORES):
        r = res.results[c]
        attn[:, 2 * c:2 * c + 2] = r["attn_o"]
        acc += r["out_o"]
    out = (acc + np.asarray(inputs["bp"], np.float64)).astype(np.float32)
    return (out, attn), res


def kernel(**inputs):
    (out, attn), _ = run(inputs, trace=False)
    return (out, attn)
